# revision 9
# baseline (speedup 1.0000x reference)
"""Causal multi-head attention on 8 Trainium2 NeuronCores.

Sharding: tensor parallel over heads. Core c owns heads {2c, 2c+1}:
  - QKV projection for its 128 q / 128 k / 128 v channels, all B*S tokens
  - causal attention for its 2 heads (row-packed K=64 matmuls)
  - partial output projection out_c = O_c @ W_o[rows of its heads]
Host side: shard/preprocess inputs (transpose x, slice + pre-scale weights),
then unshard by summing the 8 partial projections (the tensor-parallel
reduce) and adding b_o.

Schedule: the four batches are software-pipelined. While batch b's
attention runs (Act-engine exp is the per-tile critical path), batch
b+1's QKV projection matmuls and batch b's previous-chunk output
projection are drip-fed into the PE queue, so the tensor engine never
idles waiting for exp. The causal mask is applied by accumulating a
-1e9 upper-triangle matmul into the scores PSUM (no vector-engine mask
multiply), scores/exp are restricted to the live query range on
diagonal tiles, and the projection PSUM is DMA'd straight to DRAM.

Problem constants (hardcoded per the harness contract):
  x [4, 2048, 1024] f32, W_qkv [1024, 3072], b_qkv [3072],
  W_o [1024, 1024], b_o [1024]; 16 heads, d_k = 64, causal.
"""

import numpy as np

N_CORES = 8
B, S, D = 4, 2048, 1024
H = 16
DK = 64
T = B * S            # 8192 tokens
HPC = H // N_CORES   # 2 heads per core
CPC = HPC * DK       # 128 channels per core per q/k/v
NB = S // 512        # 4 q-chunks of 512 per batch
NK = S // 128        # 16 k-tiles of 128 per batch

_PROFILE = False     # test.py may set kernel._PROFILE = True
_TRACE_DIR = None
_LAST_RESULT = None  # BassKernelResults of the last run (for test.py)

_PROGRAM = None      # cached program across calls

_QLO_RESTRICT = True  # restrict scores/exp to live query range on diag tiles
_DEBUG = False        # add den/rec/otn debug outputs


def _build_program():
    import concourse.bacc as bacc
    import concourse.tile as tile
    from concourse import mybir

    F32 = mybir.dt.float32
    BF16 = mybir.dt.bfloat16
    AF = mybir.ActivationFunctionType

    nc = bacc.Bacc("TRN2", num_devices=N_CORES)

    # ---- DRAM parameters (per core) ----
    xT = nc.declare_dram_parameter("xT", [D, T], BF16, isOutput=False)
    wq = nc.declare_dram_parameter("wq", [128, 8 * CPC], BF16, isOutput=False)
    wk = nc.declare_dram_parameter("wk", [128, 8 * CPC], BF16, isOutput=False)
    wv = nc.declare_dram_parameter("wv", [128, 8 * CPC], BF16, isOutput=False)
    bq = nc.declare_dram_parameter("bq", [CPC, 1], F32, isOutput=False)
    wo = nc.declare_dram_parameter("wo", [CPC, D], BF16, isOutput=False)
    ident_d = nc.declare_dram_parameter("ident", [128, 128], BF16, isOutput=False)
    ltri_d = nc.declare_dram_parameter("ltri", [128, 2 * 128], BF16, isOutput=False)
    ones_d = nc.declare_dram_parameter("ones", [128, 1], BF16, isOutput=False)
    out = nc.declare_dram_parameter("out", [T, D], BF16, isOutput=True)
    if _DEBUG:
        dbg_den = nc.declare_dram_parameter("dbg_den", [32, 512], F32, isOutput=True)
        dbg_rec = nc.declare_dram_parameter("dbg_rec", [32, 512], F32, isOutput=True)
        dbg_otn = nc.declare_dram_parameter("dbg_otn", [CPC, T], BF16, isOutput=True)

    xT_t = xT.rearrange("(m p) t -> p m t", p=128)      # [128, 8, 8192]
    wq_t = wq.rearrange("p (m c) -> p m c", c=CPC)      # [128, 8, 128] contig
    wk_t = wk.rearrange("p (m c) -> p m c", c=CPC)
    wv_t = wv.rearrange("p (m c) -> p m c", c=CPC)

    with tile.TileContext(nc) as tc:
        with (
            tc.tile_pool(name="const", bufs=1) as const,
            tc.tile_pool(name="xt", bufs=4) as xt_pool,
            tc.tile_pool(name="qk", bufs=2) as qk_pool,
            tc.tile_pool(name="vt", bufs=3) as vt_pool,
            tc.tile_pool(name="pp", bufs=6) as pp_pool,
            tc.tile_pool(name="otn", bufs=2) as otn_pool,
            tc.tile_pool(name="osb", bufs=6) as osb_pool,
            tc.tile_pool(name="nrm", bufs=3) as nrm_pool,
            tc.tile_pool(name="ps", bufs=2, space="PSUM") as ps,
        ):
            # ---- PE p-state warmup: junk matmuls while the first DMAs are
            # in flight, so the clock is at full speed when real work lands.
            warm = const.tile([128, 640], BF16, tag="warm")
            nc.vector.memset(warm[:], 0.0)
            warm_ps = ps.tile([128, 2, 512], F32, tag="psw", name="warm_ps")
            for _ in range(12):
                nc.tensor.matmul(warm_ps[:, 0, :], warm[:, 0:128],
                                 warm[:, 128:640], start=True, stop=True)

            # ---- constants (QKV weights first so warmup can start ASAP;
            # the rest of the consts are DMA'd after warmup's first x load) ----
            wq_sb = const.tile([128, 8, CPC], BF16, tag="wq")
            nc.sync.dma_start(wq_sb[:], wq_t)
            wk_sb = const.tile([128, 8, CPC], BF16, tag="wk")
            nc.sync.dma_start(wk_sb[:], wk_t)
            wv_sb = const.tile([128, 8, CPC], BF16, tag="wv")
            nc.sync.dma_start(wv_sb[:], wv_t)
            bq_sb = const.tile([CPC, 1], F32, tag="bq")
            wo_sb = const.tile([CPC, D], BF16, tag="wo")
            ident = const.tile([128, 128], BF16, tag="ident")
            ltri = const.tile([128, 2, 128], BF16, tag="ltri")
            ones_sb = const.tile([128, 1], BF16, tag="ones")

            def emit_const_dmas():
                nc.sync.dma_start(bq_sb[:], bq[:])
                nc.sync.dma_start(ident[:], ident_d[:])
                nc.sync.dma_start(ltri[:], ltri_d.rearrange("p (g c) -> p g c", c=128))
                nc.sync.dma_start(ones_sb[:], ones_d[:])
                nc.sync.dma_start(wo_sb[:], wo[:])

            # persistent V^T tiles [V_A | 1 | V_B | 1], two sets (batch parity)
            vaug = [
                [
                    const.tile([128, 2 * (DK + 1)], BF16,
                               tag=f"vaug{st}_{j}", name=f"vaug{st}_{j}")
                    for j in range(NK)
                ]
                for st in range(2)
            ]
            def emit_persistent_init():
                # on GpSimd so the DVE is free for warmup QKV evictions
                for st in range(2):
                    for j in range(NK):
                        va_g = vaug[st][j].rearrange("p (g c) -> p g c", c=DK + 1)
                        nc.gpsimd.tensor_copy(
                            va_g[:, :, DK : DK + 1],
                            ones_sb[:, None, :].to_broadcast([128, 2, 1]))

            qt_sb = {}
            kt_sb = {}
            otn = {}
            x_tiles = {}

            def prefetch_x(b2, ch):
                t0 = b2 * S
                c0 = ch * 512
                x_sb = xt_pool.tile([128, 8, 512], BF16, tag="x",
                                    name=f"x_{b2}_{ch}")
                nc.sync.dma_start(x_sb[:], xT_t[:, :, t0 + c0 : t0 + c0 + 512])
                x_tiles[(b2, ch)] = x_sb

            def emit_qkv_units(b2, ch):
                """PE/DVE work units for QKV projection chunk (b2, ch)."""
                st = b2 % 2
                c0 = ch * 512
                units = []
                if ch == 0:
                    def alloc_qkt():
                        qt_sb[b2] = qk_pool.tile(
                            [CPC, S], BF16, tag="qt", name=f"qt_{b2}")
                        kt_sb[b2] = qk_pool.tile(
                            [CPC, S], BF16, tag="kt", name=f"kt_{b2}")
                    units.append(alloc_qkt)

                state = {}

                def take_x():
                    state["x"] = x_tiles.pop((b2, ch))
                units.append(take_x)

                def mm_group(w_sb, key, lo, hi):
                    def go():
                        if key not in state:
                            state[key] = ps.tile([CPC, 512], F32, tag="pk",
                                                 name=f"ps_{key}_{b2}_{ch}")
                        for m in range(lo, hi):
                            nc.tensor.matmul(state[key][:], w_sb[:, m, :],
                                             state["x"][:, m, :],
                                             start=(m == 0), stop=(m == 7))
                    return go

                def evict_q():
                    nc.vector.tensor_scalar_add(
                        qt_sb[b2][:, c0 : c0 + 512], state["q"][:], bq_sb[:])

                def evict_k():
                    # k bias dropped: it only adds a per-query constant to the
                    # scores, which softmax normalizes away.
                    nc.vector.tensor_copy(
                        kt_sb[b2][:, c0 : c0 + 512], state["k"][:])

                units.append(mm_group(wq_sb, "q", 0, 4))
                units.append(mm_group(wq_sb, "q", 4, 8))
                units.append(evict_q)
                units.append(mm_group(wk_sb, "k", 0, 4))
                units.append(mm_group(wk_sb, "k", 4, 8))
                units.append(evict_k)
                units.append(mm_group(wv_sb, "v", 0, 4))
                units.append(mm_group(wv_sb, "v", 4, 8))

                def evict_v():
                    # v bias dropped: softmax weights sum to 1, so b_v's
                    # contribution is a constant folded into b_o host-side.
                    vtmp = vt_pool.tile([CPC, 512], BF16, tag="vtmp",
                                        name=f"vtmp_{b2}_{ch}")
                    nc.vector.tensor_copy(vtmp[:], state["v"][:])
                    state["vtmp"] = vtmp
                units.append(evict_v)

                def transp(jj):
                    def go():
                        j = ch * 4 + jj
                        ps_t = ps.tile([128, 128], BF16, tag="pk",
                                       name=f"ps_t_{b2}_{ch}_{jj}")
                        nc.tensor.transpose(
                            ps_t[:], state["vtmp"][:, jj * 128 : jj * 128 + 128],
                            ident[:])
                        va_g = vaug[st][j].rearrange("p (g c) -> p g c", c=DK + 1)
                        pt_g = ps_t.rearrange("p (g c) -> p g c", c=DK)
                        # on Act (scalar) to keep the DVE free for evictions
                        nc.scalar.copy(va_g[:, :, 0:DK], pt_g[:])
                    return go

                for jj in range(4):
                    units.append(transp(jj))
                return units

            def emit_proj_units(b, ch):
                """Output projection for chunk (b, ch). bf16 eviction split
                across DVE and GpSimd, one DMA per 128-token tile."""
                t0 = b * S
                c0 = ch * 512
                units = []

                def proj_mm(tt, state):
                    def go():
                        q0 = c0 + tt * 128
                        for half in range(2):
                            p_h = ps.tile([128, 512], F32, tag="pk",
                                          name=f"ps_o_{b}_{ch}_{tt}_{half}")
                            nc.tensor.matmul(p_h[:],
                                             otn[b][:, q0 : q0 + 128],
                                             wo_sb[:, half * 512 : half * 512 + 512],
                                             start=True, stop=True)
                            state[half] = p_h
                    return go

                def proj_out(tt, state):
                    def go():
                        q0 = c0 + tt * 128
                        o_sb = osb_pool.tile([128, D], BF16, tag="osb",
                                             name=f"o_sb_{b}_{ch}_{tt}")
                        nc.vector.tensor_copy(o_sb[:, 0:512], state[0][:])
                        if b <= 1 or tt % 2 == 0:
                            nc.scalar.copy(o_sb[:, 512:1024], state[1][:])
                        else:
                            nc.vector.tensor_copy(o_sb[:, 512:1024], state[1][:])
                        nc.sync.dma_start(out[t0 + q0 : t0 + q0 + 128, :], o_sb[:])
                    return go

                for tt in range(4):
                    state = {}
                    units.append(proj_mm(tt, state))
                    units.append(proj_out(tt, state))
                return units

            def emit_attention(b, ch, units):
                """Attention for chunk (b, ch), draining `units` between tiles."""
                st = b % 2
                c0 = ch * 512
                jmax = ch * 4 + 3
                ntiles = jmax + 1
                av_a = ps.tile([DK + 1, 512], F32, tag="av", name=f"av_a_{b}_{ch}")
                av_b = ps.tile([DK + 1, 512], F32, tag="av", name=f"av_b_{b}_{ch}")
                psel = [None] * ntiles
                plo = [0] * ntiles

                def emit_av(j):
                    first, last = j == 0, j == jmax
                    lo = plo[j]
                    nc.tensor.matmul(av_a[:, lo:512], vaug[st][j][:, 0 : DK + 1],
                                     psel[j][:, 0, lo:512], start=first, stop=last)
                    nc.tensor.matmul(av_b[:, lo:512],
                                     vaug[st][j][:, DK + 1 : 2 * DK + 2],
                                     psel[j][:, 1, lo:512], start=first, stop=last)

                # distribute unit drains across tiles
                drains = [0] * (ntiles + 1)
                for u in range(len(units)):
                    drains[(u * ntiles) // len(units)] += 1
                ui = 0

                for j in range(ntiles):
                    r = j - ch * 4
                    k0 = j * 128
                    q_lo = (128 * r if r >= 1 else 0) if _QLO_RESTRICT else 0
                    s_ab = ps.tile([128, 2, 512], F32, tag="psw",
                                   name=f"s_ab_{b}_{ch}_{j}")
                    nc.tensor.matmul(
                        s_ab[:, 0, q_lo:512], kt_sb[b][0:DK, k0 : k0 + 128],
                        qt_sb[b][0:DK, c0 + q_lo : c0 + 512],
                        start=True, stop=True, tile_position=(0, 0),
                    )
                    nc.tensor.matmul(
                        s_ab[:, 1, q_lo:512], kt_sb[b][DK:128, k0 : k0 + 128],
                        qt_sb[b][DK:128, c0 + q_lo : c0 + 512],
                        start=True, stop=True, tile_position=(64, 0),
                    )
                    psel[j] = pp_pool.tile([128, 2, 512], BF16, tag="pt",
                                           name=f"p_{b}_{ch}_{j}")
                    plo[j] = q_lo
                    nc.scalar.activation(psel[j][:, :, q_lo:512],
                                         s_ab[:, :, q_lo:512], AF.Exp)
                    if r >= 0:
                        # causal mask on the 128-wide diagonal block only
                        # (everything right of it is fully unmasked).
                        d0 = 128 * r
                        pm = psel[j][:, :, d0 : d0 + 128]
                        nc.vector.tensor_mul(pm, pm, ltri[:])
                        if r >= 1 and not _QLO_RESTRICT:
                            # fully-masked queries left of the diagonal block
                            nc.vector.memset(psel[j][:, :, 0:d0], 0.0)
                    if j >= 1:
                        emit_av(j - 1)
                    for _ in range(drains[j]):
                        units[ui]()
                        ui += 1
                emit_av(jmax)

                # normalize: otn[h] = av[0:64] * (1 / av[64]).
                # reciprocal_approx_fast must NOT read PSUM directly, and
                # needs a partition-0 input (custom-DVE op): stage den
                # through a [1,512] SBUF tile.
                for hh, av in ((0, av_a), (1, av_b)):
                    den = nrm_pool.tile([1, 512], F32, tag="den",
                                        name=f"den_{b}_{ch}_{hh}")
                    nc.vector.tensor_copy(den[:], av[DK : DK + 1, :])
                    rec = nrm_pool.tile([1, 512], F32, tag="rec",
                                        name=f"rec_{b}_{ch}_{hh}")
                    nc.vector.reciprocal_approx_fast(rec[:], den[:])
                    bc = nrm_pool.tile([DK, 512], F32, tag="bc",
                                       name=f"bc_{b}_{ch}_{hh}")
                    nc.gpsimd.partition_broadcast(bc[:], rec[:])
                    nc.vector.tensor_mul(
                        otn[b][hh * DK : hh * DK + DK, c0 : c0 + 512],
                        av[0:DK, :], bc[:],
                    )
                    if _DEBUG:
                        row = (b * NB + ch) * 2 + hh
                        den_sb = nrm_pool.tile([1, 512], F32, tag="dbgden",
                                               bufs=4, name=f"den_{b}_{ch}_{hh}")
                        nc.vector.tensor_copy(den_sb[:], av[DK : DK + 1, :])
                        nc.sync.dma_start(dbg_den[row : row + 1, :], den_sb[:])
                        nc.sync.dma_start(dbg_rec[row : row + 1, :], rec[:])
                if _DEBUG and ch == NB - 1:
                    nc.sync.dma_start(dbg_otn[:, b * S : (b + 1) * S], otn[b][:])
                while ui < len(units):
                    units[ui]()
                    ui += 1

            # x-chunk prefetch: one slot of lead over the QKV consumption.
            # QKV consumption order: (0,0) in the (short) warmup, then batch
            # 0's remaining chunks ride inside the batch-0 attention slots
            # alongside batch 1's.
            seq = [(0, c) for c in range(NB)] + [
                (b + 1, c) for b in range(B - 1) for c in range(NB)]
            pf = {"i": 0}

            def pump(n=1):
                for _ in range(n):
                    if pf["i"] < len(seq):
                        prefetch_x(*seq[pf["i"]])
                        pf["i"] += 1

            # ---- warmup: QKV for batch 0 ----
            pump(3)
            for ch in range(NB):
                for i, u in enumerate(emit_qkv_units(0, ch)):
                    u()
                    if ch == 0 and i == 0:
                        emit_const_dmas()
                if ch == 0:
                    emit_persistent_init()
                else:
                    pump()

            # ---- pipelined main loop ----
            # proj chunks go through a FIFO lagged one full batch: batch b's
            # projections drain inside batch b+1's slots.  b=3's slots carry 2
            # chunks each (they have no QKV work left to keep the PE busy),
            # leaving a single chunk for the tail drain.
            pending = []
            for b in range(B):
                otn[b] = otn_pool.tile([CPC, S], BF16, tag="otn", name=f"otn_{b}")
                for ch in range(NB):
                    units = []
                    if b + 1 < B:
                        units += emit_qkv_units(b + 1, ch)
                        pump()
                    ndrain = {0: 0, 1: 1, 2: 1, 3: 2}[b]
                    for _ in range(ndrain):
                        if pending:
                            units += emit_proj_units(*pending.pop(0))
                    emit_attention(b, ch, units)
                    pending.append((b, ch))
            while pending:
                for u in emit_proj_units(*pending.pop(0)):
                    u()

    nc.compile()
    return nc


def _get_program():
    global _PROGRAM
    if _PROGRAM is None:
        _PROGRAM = _build_program()
    return _PROGRAM


def kernel(x, W_qkv, b_qkv, W_o, b_o):
    global _LAST_RESULT
    from concourse.bass_utils import run_bass_kernel_spmd

    x = np.asarray(x, np.float32)
    W_qkv = np.asarray(W_qkv, np.float32)
    b_qkv = np.asarray(b_qkv, np.float32)
    W_o = np.asarray(W_o, np.float32)
    b_o = np.asarray(b_o, np.float32)

    # host-side shard/preprocess
    import ml_dtypes
    bf16 = ml_dtypes.bfloat16
    xT = np.ascontiguousarray(x.reshape(T, D).T).astype(bf16)   # [1024, 8192]
    scale = np.float32(1.0 / np.sqrt(DK))
    ident = np.eye(128, dtype=bf16)
    ones = np.ones((128, 1), bf16)
    # keep-mask for the 128-wide causal diagonal block: kk <= qq,
    # duplicated side by side for the two heads' score planes
    ltri1 = (np.arange(128)[:, None] <= np.arange(128)[None, :]).astype(bf16)
    ltri = np.ascontiguousarray(np.concatenate([ltri1, ltri1], axis=1))

    def _wprep(w):
        # [1024, 128] -> [128, 8*128]: per-partition contiguous DMA layout
        return np.ascontiguousarray(
            w.reshape(8, 128, CPC).transpose(1, 0, 2).reshape(128, 8 * CPC)
        ).astype(bf16)

    in_maps = []
    for c in range(N_CORES):
        cs = c * CPC
        in_maps.append({
            "xT": xT,
            "wq": _wprep(W_qkv[:, cs : cs + CPC] * scale),
            "wk": _wprep(W_qkv[:, D + cs : D + cs + CPC]),
            "wv": _wprep(W_qkv[:, 2 * D + cs : 2 * D + cs + CPC]),
            "bq": np.ascontiguousarray(b_qkv[cs : cs + CPC, None] * scale),
            "wo": np.ascontiguousarray(W_o[cs : cs + CPC, :]).astype(bf16),
            "ident": ident,
            "ltri": ltri,
            "ones": ones,
        })

    nc = _get_program()
    res = run_bass_kernel_spmd(
        nc, in_maps, list(range(N_CORES)),
        trace=_PROFILE, tmpdir=_TRACE_DIR,
    )
    _LAST_RESULT = res

    # unshard: tensor-parallel reduce of the 8 partial projections, plus
    # b_o and the folded v-bias contribution (softmax weights sum to 1, so
    # the attention output of v + b_v is the output of v plus b_v exactly).
    acc = res.results[0]["out"].astype(np.float32)
    for c in range(1, N_CORES):
        acc += res.results[c]["out"]
    acc += b_o[None, :] + b_qkv[2 * D : 3 * D] @ W_o
    return acc.reshape(B, S, D)



# revision 49
# speedup vs baseline: 1.0891x; 1.0891x over previous
"""Causal multi-head attention on 8 Trainium2 NeuronCores.

Sharding: tensor parallel over heads. Core c owns heads {2c, 2c+1}:
  - QKV projection for its 128 q / 128 k / 128 v channels, all B*S tokens
  - causal attention for its 2 heads (row-packed K=64 matmuls)
  - partial output projection out_c = O_c @ W_o[rows of its heads]
Host side: shard/preprocess inputs (transpose x, slice + pre-scale weights),
then unshard by summing the 8 partial projections (the tensor-parallel
reduce) and adding b_o.

Schedule: the four batches are software-pipelined. While batch b's
attention runs (Act-engine exp is the per-tile critical path), batch
b+1's QKV projection matmuls and batch b's one-batch-lagged output
projection are drip-fed into the PE queue, so the tensor engine never
idles waiting for exp.  AV matmuls lag their exp by three k-tiles (slack
for the exp and for the av psum bank to rotate).  The k/v biases are
dropped (k's is softmax-invariant; v's folds into b_o host-side), exp
input is restricted to the live query range on diagonal tiles, and
batch 3 runs its chunks [1,2,3,0] so the two kept-back projection
chunks fill the PE during the final norm chain.

Problem constants (hardcoded per the harness contract):
  x [4, 2048, 1024] f32, W_qkv [1024, 3072], b_qkv [3072],
  W_o [1024, 1024], b_o [1024]; 16 heads, d_k = 64, causal.
"""

import numpy as np

N_CORES = 8
B, S, D = 4, 2048, 1024
H = 16
DK = 64
T = B * S            # 8192 tokens
HPC = H // N_CORES   # 2 heads per core
CPC = HPC * DK       # 128 channels per core per q/k/v
NB = S // 512        # 4 q-chunks of 512 per batch
NK = S // 128        # 16 k-tiles of 128 per batch

_PROFILE = False     # test.py may set kernel._PROFILE = True
_TRACE_DIR = None
_LAST_RESULT = None  # BassKernelResults of the last run (for test.py)

_PROGRAM = None      # cached program across calls

_QLO_RESTRICT = True  # restrict scores/exp to live query range on diag tiles
_DEBUG = False        # add den/rec/otn debug outputs
_PE_MASK = False    # causal mask via -1e9 PE accumulate (else DVE multiply)
_DMA_BCAST = False  # broadcast 1/den across partitions via DMA (unsupported)


def _build_program():
    import concourse.bacc as bacc
    import concourse.tile as tile
    from concourse import mybir

    F32 = mybir.dt.float32
    BF16 = mybir.dt.bfloat16
    AF = mybir.ActivationFunctionType

    nc = bacc.Bacc("TRN2", num_devices=N_CORES)

    # ---- DRAM parameters (per core) ----
    xT = nc.declare_dram_parameter("xT", [D, T], BF16, isOutput=False)
    wq = nc.declare_dram_parameter("wq", [128, 8 * CPC], BF16, isOutput=False)
    wk = nc.declare_dram_parameter("wk", [128, 8 * CPC], BF16, isOutput=False)
    wv = nc.declare_dram_parameter("wv", [128, 8 * CPC], BF16, isOutput=False)
    bq = nc.declare_dram_parameter("bq", [CPC, 1], F32, isOutput=False)
    wo = nc.declare_dram_parameter("wo", [CPC, D], BF16, isOutput=False)
    ident_d = nc.declare_dram_parameter("ident", [128, 128], BF16, isOutput=False)
    negI_d = nc.declare_dram_parameter("negI", [128, 128], BF16, isOutput=False)
    triu_d = nc.declare_dram_parameter("triu", [128, 2 * 128], BF16, isOutput=False)
    ones_d = nc.declare_dram_parameter("ones", [128, 1], BF16, isOutput=False)
    out = nc.declare_dram_parameter("out", [T, D], BF16, isOutput=True)
    if _DEBUG:
        dbg_den = nc.declare_dram_parameter("dbg_den", [32, 512], F32, isOutput=True)
        dbg_rec = nc.declare_dram_parameter("dbg_rec", [32, 512], F32, isOutput=True)
        dbg_otn = nc.declare_dram_parameter("dbg_otn", [CPC, T], BF16, isOutput=True)

    xT_t = xT.rearrange("(m p) t -> p m t", p=128)      # [128, 8, 8192]
    wq_t = wq.rearrange("p (m c) -> p m c", c=CPC)      # [128, 8, 128] contig
    wk_t = wk.rearrange("p (m c) -> p m c", c=CPC)
    wv_t = wv.rearrange("p (m c) -> p m c", c=CPC)

    with tile.TileContext(nc) as tc:
        with (
            tc.tile_pool(name="const", bufs=1) as const,
            tc.tile_pool(name="xt", bufs=4) as xt_pool,
            tc.tile_pool(name="qk", bufs=2) as qk_pool,
            tc.tile_pool(name="vt", bufs=3) as vt_pool,
            tc.tile_pool(name="pp", bufs=6) as pp_pool,
            tc.tile_pool(name="otn", bufs=3) as otn_pool,
            tc.tile_pool(name="osb", bufs=6) as osb_pool,
            tc.tile_pool(name="nrm", bufs=3) as nrm_pool,
            tc.tile_pool(name="ps", bufs=2, space="PSUM") as ps,
        ):
            # ---- PE p-state warmup: junk matmuls while the first DMAs are
            # in flight, so the clock is at full speed when real work lands.
            warm = const.tile([128, 640], BF16, tag="warm")
            nc.vector.memset(warm[:], 0.0)
            warm_ps = ps.tile([128, 2, 512], F32, tag="psw", name="warm_ps")
            for _ in range(8):
                nc.tensor.matmul(warm_ps[:, 0, :], warm[:, 0:128],
                                 warm[:, 128:640], start=True, stop=True)

            # ---- constants (x chunk 0 + wq are DMA'd first, below, so the
            # first q matmuls can start ASAP; the rest follow) ----
            wq_sb = const.tile([128, 8, CPC], BF16, tag="wq")
            wk_sb = const.tile([128, 8, CPC], BF16, tag="wk")
            wv_sb = const.tile([128, 8, CPC], BF16, tag="wv")
            bq_sb = const.tile([CPC, 1], F32, tag="bq")
            wo_sb = const.tile([CPC, D], BF16, tag="wo")
            ident = const.tile([128, 128], BF16, tag="ident")
            negI = const.tile([128, 128], BF16, tag="negI")
            triu = const.tile([128, 2, 128], BF16, tag="triu")
            ones_sb = const.tile([128, 1], BF16, tag="ones")

            def emit_const_dmas():
                nc.sync.dma_start(bq_sb[:], bq[:])
                nc.sync.dma_start(ident[:], ident_d[:])
                nc.sync.dma_start(negI[:], negI_d[:])
                nc.sync.dma_start(triu[:], triu_d.rearrange("p (g c) -> p g c", c=128))
                nc.sync.dma_start(ones_sb[:], ones_d[:])
                nc.sync.dma_start(wo_sb[:], wo[:])

            # persistent V^T tiles [V_A | 1 | V_B | 1], two sets (batch parity)
            vaug = [
                [
                    const.tile([128, 2 * (DK + 1)], BF16,
                               tag=f"vaug{st}_{j}", name=f"vaug{st}_{j}")
                    for j in range(NK)
                ]
                for st in range(2)
            ]
            def emit_persistent_init():
                # on GpSimd so the DVE is free for warmup QKV evictions
                for st in range(2):
                    for j in range(NK):
                        va_g = vaug[st][j].rearrange("p (g c) -> p g c", c=DK + 1)
                        nc.gpsimd.tensor_copy(
                            va_g[:, :, DK : DK + 1],
                            ones_sb[:, None, :].to_broadcast([128, 2, 1]))

            qt_sb = {}
            kt_sb = {}
            otn = {}
            x_tiles = {}

            def prefetch_x(b2, ch, split=False):
                t0 = b2 * S
                c0 = ch * 512
                x_sb = xt_pool.tile([128, 8, 512], BF16, tag="x",
                                    name=f"x_{b2}_{ch}")
                sl = xT_t[:, :, t0 + c0 : t0 + c0 + 512]
                if split:
                    # two half-DMAs so the first m-groups can start as soon
                    # as the first half lands (startup latency)
                    nc.sync.dma_start(x_sb[:, 0:4, :], sl[:, 0:4, :])
                    nc.sync.dma_start(x_sb[:, 4:8, :], sl[:, 4:8, :])
                else:
                    nc.sync.dma_start(x_sb[:], sl)
                x_tiles[(b2, ch)] = x_sb

            def emit_qkv_units(b2, ch):
                """PE/DVE work units for QKV projection chunk (b2, ch)."""
                st = b2 % 2
                c0 = ch * 512
                units = []
                if ch == 0:
                    def alloc_qkt():
                        qt_sb[b2] = qk_pool.tile(
                            [CPC, S], BF16, tag="qt", name=f"qt_{b2}")
                        kt_sb[b2] = qk_pool.tile(
                            [CPC, S], BF16, tag="kt", name=f"kt_{b2}")
                    units.append(alloc_qkt)

                state = {}

                def take_x():
                    state["x"] = x_tiles.pop((b2, ch))
                units.append(take_x)

                def mm_group(w_sb, key, lo, hi):
                    def go():
                        if key not in state:
                            state[key] = ps.tile([CPC, 512], F32, tag="pk",
                                                 name=f"ps_{key}_{b2}_{ch}")
                        for m in range(lo, hi):
                            nc.tensor.matmul(state[key][:], w_sb[:, m, :],
                                             state["x"][:, m, :],
                                             start=(m == 0), stop=(m == 7))
                    return go

                def evict_q():
                    nc.vector.tensor_scalar_add(
                        qt_sb[b2][:, c0 : c0 + 512], state["q"][:], bq_sb[:])

                def evict_k():
                    # k bias dropped: it only adds a per-query constant to the
                    # scores, which softmax normalizes away.
                    nc.vector.tensor_copy(
                        kt_sb[b2][:, c0 : c0 + 512], state["k"][:])

                units.append(mm_group(wq_sb, "q", 0, 4))
                units.append(mm_group(wq_sb, "q", 4, 8))
                units.append(evict_q)
                units.append(mm_group(wk_sb, "k", 0, 4))
                units.append(mm_group(wk_sb, "k", 4, 8))
                units.append(evict_k)
                units.append(mm_group(wv_sb, "v", 0, 4))
                units.append(mm_group(wv_sb, "v", 4, 8))

                def evict_v():
                    # v bias dropped: softmax weights sum to 1, so b_v's
                    # contribution is a constant folded into b_o host-side.
                    vtmp = vt_pool.tile([CPC, 512], BF16, tag="vtmp",
                                        name=f"vtmp_{b2}_{ch}")
                    nc.vector.tensor_copy(vtmp[:], state["v"][:])
                    state["vtmp"] = vtmp
                units.append(evict_v)

                def transp(jj):
                    def go():
                        j = ch * 4 + jj
                        ps_t = ps.tile([128, 128], BF16, tag="pk",
                                       name=f"ps_t_{b2}_{ch}_{jj}")
                        nc.tensor.transpose(
                            ps_t[:], state["vtmp"][:, jj * 128 : jj * 128 + 128],
                            ident[:])
                        va_g = vaug[st][j].rearrange("p (g c) -> p g c", c=DK + 1)
                        pt_g = ps_t.rearrange("p (g c) -> p g c", c=DK)
                        if b2 == 0:
                            # warmup phase: DVE is busy with q/k/v evicts and
                            # Act is idle, so evict the transposes there
                            nc.scalar.copy(va_g[:, :, 0:DK], pt_g[:])
                        else:
                            nc.vector.tensor_copy(va_g[:, :, 0:DK], pt_g[:])
                    return go

                for jj in range(4):
                    units.append(transp(jj))
                return units

            def emit_proj_units(b, ch, act_evict=False):
                """Output projection for chunk (b, ch). bf16 eviction split
                across DVE and Act, one DMA per 128-token tile."""
                t0 = b * S
                c0 = ch * 512
                units = []

                def proj_mm(tt, state):
                    def go():
                        q0 = c0 + tt * 128
                        for half in range(2):
                            p_h = ps.tile([128, 512], F32, tag="pk",
                                          name=f"ps_o_{b}_{ch}_{tt}_{half}")
                            nc.tensor.matmul(p_h[:],
                                             otn[b][:, q0 : q0 + 128],
                                             wo_sb[:, half * 512 : half * 512 + 512],
                                             start=True, stop=True)
                            state[half] = p_h
                    return go

                def proj_out(tt, state):
                    def go():
                        q0 = c0 + tt * 128
                        o_sb = osb_pool.tile([128, D], BF16, tag="osb",
                                             name=f"o_sb_{b}_{ch}_{tt}")
                        if act_evict:
                            # tail chunks: both halves on Act, keeping the
                            # DVE free for the final norm chain
                            nc.scalar.copy(o_sb[:, 0:512], state[0][:])
                            nc.scalar.copy(o_sb[:, 512:1024], state[1][:])
                        else:
                            nc.vector.tensor_copy(o_sb[:, 0:512], state[0][:])
                            if b <= 1 or tt % 2 == 0:
                                nc.scalar.copy(o_sb[:, 512:1024], state[1][:])
                            else:
                                nc.vector.tensor_copy(o_sb[:, 512:1024], state[1][:])
                        nc.sync.dma_start(out[t0 + q0 : t0 + q0 + 128, :], o_sb[:])
                    return go

                for tt in range(4):
                    state = {}
                    units.append(proj_mm(tt, state))
                    units.append(proj_out(tt, state))
                return units

            def emit_attention(b, ch, units, tail_units=()):
                """Attention for chunk (b, ch), draining `units` between
                tiles; `tail_units` are emitted between the last AV and the
                norm so they fill the PE during the final norm chain."""
                st = b % 2
                c0 = ch * 512
                jmax = ch * 4 + 3
                ntiles = jmax + 1
                av_a = ps.tile([DK + 1, 512], F32, tag="av", name=f"av_a_{b}_{ch}")
                av_b = ps.tile([DK + 1, 512], F32, tag="av", name=f"av_b_{b}_{ch}")
                psel = [None] * ntiles
                plo = [0] * ntiles

                def emit_av(j):
                    first, last = j == 0, j == jmax
                    lo = plo[j]
                    nc.tensor.matmul(av_a[:, lo:512], vaug[st][j][:, 0 : DK + 1],
                                     psel[j][:, 0, lo:512], start=first, stop=last)
                    nc.tensor.matmul(av_b[:, lo:512],
                                     vaug[st][j][:, DK + 1 : 2 * DK + 2],
                                     psel[j][:, 1, lo:512], start=first, stop=last)

                # distribute unit drains across tiles
                drains = [0] * (ntiles + 1)
                for u in range(len(units)):
                    drains[(u * ntiles) // len(units)] += 1
                ui = 0

                for j in range(ntiles):
                    r = j - ch * 4
                    k0 = j * 128
                    q_lo = (128 * r if r >= 1 else 0) if _QLO_RESTRICT else 0
                    diag = r >= 0
                    mmask = diag and _PE_MASK
                    s_ab = ps.tile([128, 2, 512], F32, tag="psw",
                                   name=f"s_ab_{b}_{ch}_{j}")
                    nc.tensor.matmul(
                        s_ab[:, 0, q_lo:512], kt_sb[b][0:DK, k0 : k0 + 128],
                        qt_sb[b][0:DK, c0 + q_lo : c0 + 512],
                        start=True, stop=not mmask, tile_position=(0, 0),
                    )
                    nc.tensor.matmul(
                        s_ab[:, 1, q_lo:512], kt_sb[b][DK:128, k0 : k0 + 128],
                        qt_sb[b][DK:128, c0 + q_lo : c0 + 512],
                        start=True, stop=not mmask, tile_position=(64, 0),
                    )
                    if mmask:
                        # causal mask: accumulate -1e9 onto the strictly-upper
                        # part of the 128-wide diagonal block so exp() zeroes
                        # it.  One full-K matmul per head plane: two row-tiled
                        # matmuls would accumulate into the same psum bank
                        # concurrently, which wedges the PE.
                        d0 = 128 * r
                        for h in range(2):
                            nc.tensor.matmul(
                                s_ab[:, h, d0 : d0 + 128], negI[:],
                                triu[:, h, :],
                                start=False, stop=True,
                            )
                    psel[j] = pp_pool.tile([128, 2, 512], BF16, tag="pt",
                                           name=f"p_{b}_{ch}_{j}")
                    plo[j] = q_lo
                    nc.scalar.activation(psel[j][:, :, q_lo:512],
                                         s_ab[:, :, q_lo:512], AF.Exp)
                    if diag and not _PE_MASK:
                        d0 = 128 * r
                        pm = psel[j][:, :, d0 : d0 + 128]
                        nc.vector.tensor_mul(pm, pm, triu[:])
                    if j >= 3:
                        # three-tile lag: ample slack for exp, and the first
                        # AV lands after the previous chunk's norm has freed
                        # the av psum bank
                        emit_av(j - 3)
                    for _ in range(drains[j]):
                        units[ui]()
                        ui += 1
                for jj in range(max(0, ntiles - 3), ntiles):
                    emit_av(jj)
                for u in tail_units:
                    u()

                # normalize: otn[h] = av[0:64] * (1 / av[64]).
                # reciprocal_approx_fast must NOT read PSUM directly, and
                # needs a partition-0 input (custom-DVE op): stage den
                # through a [1,512] SBUF tile.
                def norm(hh, av, cl, w):
                    den = nrm_pool.tile([1, 512], F32, tag="den",
                                        name=f"den_{b}_{ch}_{hh}_{cl}")
                    nc.vector.tensor_copy(den[:, 0:w], av[DK : DK + 1, cl : cl + w])
                    rec = nrm_pool.tile([1, 512], F32, tag="rec",
                                        name=f"rec_{b}_{ch}_{hh}_{cl}")
                    nc.vector.reciprocal_approx_fast(rec[:, 0:w], den[:, 0:w])
                    bc = nrm_pool.tile([DK, 512], F32, tag="bc",
                                       name=f"bc_{b}_{ch}_{hh}_{cl}")
                    nc.gpsimd.partition_broadcast(bc[:, 0:w], rec[:, 0:w])
                    nc.vector.tensor_mul(
                        otn[b][hh * DK : hh * DK + DK, c0 + cl : c0 + cl + w],
                        av[0:DK, cl : cl + w], bc[:, 0:w],
                    )

                for hh, av in ((0, av_a), (1, av_b)):
                    norm(hh, av, 0, 512)
                if _DEBUG and ch == NB - 1:
                    nc.sync.dma_start(dbg_otn[:, b * S : (b + 1) * S], otn[b][:])
                while ui < len(units):
                    units[ui]()
                    ui += 1

            # x-chunk prefetch: one slot of lead over the QKV consumption.
            # QKV consumption order: (0,0) in the (short) warmup, then batch
            # 0's remaining chunks ride inside the batch-0 attention slots
            # alongside batch 1's.
            seq = [(0, c) for c in range(NB)] + [
                (b + 1, c) for b in range(B - 1) for c in range(NB)]
            pf = {"i": 1}

            def pump(n=1):
                for _ in range(n):
                    if pf["i"] < len(seq):
                        prefetch_x(*seq[pf["i"]])
                        pf["i"] += 1

            # ---- warmup: QKV for batch 0 ----
            # DMA order: x(0,0) first half, wq, x(0,0) second half, wk, wv,
            # then the next x chunks — so the first q matmuls are unblocked
            # as early as possible.  The V transposes of each chunk are
            # deferred past the next chunk's q matmuls so the PE never waits
            # on the DVE eviction chain.
            # x halves on the SP queue, weights on the Act queue: dispatches
            # and transfers run in parallel on separate DMA queues
            x00 = xt_pool.tile([128, 8, 512], BF16, tag="x", name="x_0_0")
            nc.sync.dma_start(x00[:, 0:4, :], xT_t[:, 0:4, 0:512])
            nc.scalar.dma_start(wq_sb[:], wq_t)
            nc.sync.dma_start(x00[:, 4:8, :], xT_t[:, 4:8, 0:512])
            nc.scalar.dma_start(wk_sb[:], wk_t)
            nc.scalar.dma_start(wv_sb[:], wv_t)
            x_tiles[(0, 0)] = x00
            pump(2)
            prev_tr = []
            for ch in range(NB):
                us = emit_qkv_units(0, ch)
                main, tr = us[:-4], us[-4:]
                for i, u in enumerate(main):
                    u()
                    if ch == 0 and i == 0:
                        emit_const_dmas()
                    if i == 3 and prev_tr:
                        # previous chunk's V transposes after this chunk's
                        # first q matmuls, so the PE isn't stuck waiting on
                        # the DVE eviction chain
                        for t in prev_tr:
                            t()
                        prev_tr = []
                if ch == 0:
                    emit_persistent_init()
                else:
                    pump()
                prev_tr = tr
            for t in prev_tr:
                t()

            # ---- pipelined main loop ----
            # proj chunks go through a FIFO lagged one full batch: batch b's
            # projections drain inside batch b+1's slots.  b=3's slots carry 2
            # chunks each (they have no QKV work left to keep the PE busy),
            # leaving a single chunk for the tail drain.
            pending = []
            for b in range(B):
                otn[b] = otn_pool.tile([CPC, S], BF16, tag="otn", name=f"otn_{b}")
                # b3 runs its chunks [1,2,3,0] so the final chunk is the
                # small one and the (3,3) projection can fill the PE during
                # the last norm chain
                chs = [1, 2, 3, 0] if b == B - 1 else range(NB)
                for k, ch in enumerate(chs):
                    last = b == B - 1 and k == NB - 1
                    units = []
                    if b + 1 < B:
                        units += emit_qkv_units(b + 1, ch)
                        pump()
                    ndrain = {0: 0, 1: 1, 2: 1, 3: 2}[b]
                    if b == B - 1:
                        # keep two chunks back as PE filler for the final
                        # norm chain
                        ndrain = [2, 2, 1, 0][k]
                    for _ in range(ndrain):
                        if pending:
                            units += emit_proj_units(*pending.pop(0))
                    tail_units = []
                    if last:
                        while pending:
                            tail_units += emit_proj_units(*pending.pop(0),
                                                          act_evict=True)
                    emit_attention(b, ch, units, tail_units)
                    pending.append((b, ch))
            while pending:
                for u in emit_proj_units(*pending.pop(0)):
                    u()

    nc.compile()
    return nc


def _get_program():
    global _PROGRAM
    if _PROGRAM is None:
        _PROGRAM = _build_program()
    return _PROGRAM


def kernel(x, W_qkv, b_qkv, W_o, b_o):
    global _LAST_RESULT
    from concourse.bass_utils import run_bass_kernel_spmd

    x = np.asarray(x, np.float32)
    W_qkv = np.asarray(W_qkv, np.float32)
    b_qkv = np.asarray(b_qkv, np.float32)
    W_o = np.asarray(W_o, np.float32)
    b_o = np.asarray(b_o, np.float32)

    # host-side shard/preprocess
    import ml_dtypes
    bf16 = ml_dtypes.bfloat16
    xT = np.ascontiguousarray(x.reshape(T, D).T).astype(bf16)   # [1024, 8192]
    scale = np.float32(1.0 / np.sqrt(DK))
    ones = np.ones((128, 1), bf16)
    ident = np.eye(128, dtype=bf16)
    # PE-side causal mask: negI @ triu accumulates -1e9 where key > query
    # on the 128-wide diagonal block (pattern duplicated for the two heads).
    # With _PE_MASK off, "triu" instead carries the keep-mask for the DVE
    # multiply fallback.
    negI = (np.float32(-1e9) * np.eye(128, dtype=np.float32)).astype(bf16)
    cmp = np.arange(128)[:, None] > np.arange(128)[None, :]
    triu1 = (cmp if _PE_MASK else ~cmp).astype(bf16)
    triu = np.ascontiguousarray(np.concatenate([triu1, triu1], axis=1))

    def _wprep(w):
        # [1024, 128] -> [128, 8*128]: per-partition contiguous DMA layout
        return np.ascontiguousarray(
            w.reshape(8, 128, CPC).transpose(1, 0, 2).reshape(128, 8 * CPC)
        ).astype(bf16)

    in_maps = []
    for c in range(N_CORES):
        cs = c * CPC
        in_maps.append({
            "xT": xT,
            "wq": _wprep(W_qkv[:, cs : cs + CPC] * scale),
            "wk": _wprep(W_qkv[:, D + cs : D + cs + CPC]),
            "wv": _wprep(W_qkv[:, 2 * D + cs : 2 * D + cs + CPC]),
            "bq": np.ascontiguousarray(b_qkv[cs : cs + CPC, None] * scale),
            "wo": np.ascontiguousarray(W_o[cs : cs + CPC, :]).astype(bf16),
            "ident": ident,
            "negI": negI,
            "triu": triu,
            "ones": ones,
        })

    nc = _get_program()
    res = run_bass_kernel_spmd(
        nc, in_maps, list(range(N_CORES)),
        trace=_PROFILE, tmpdir=_TRACE_DIR,
    )
    _LAST_RESULT = res

    # unshard: tensor-parallel reduce of the 8 partial projections, plus
    # b_o and the folded v-bias contribution (softmax weights sum to 1, so
    # the attention output of v + b_v is the output of v plus b_v exactly).
    acc = res.results[0]["out"].astype(np.float32)
    for c in range(1, N_CORES):
        acc += res.results[c]["out"]
    acc += b_o[None, :] + b_qkv[2 * D : 3 * D] @ W_o
    return acc.reshape(B, S, D)



# revision 50
# speedup vs baseline: 1.0963x; 1.0066x over previous
"""Causal multi-head attention on 8 Trainium2 NeuronCores.

Sharding: tensor parallel over heads. Core c owns heads {2c, 2c+1}:
  - QKV projection for its 128 q / 128 k / 128 v channels, all B*S tokens
  - causal attention for its 2 heads (row-packed K=64 matmuls)
  - partial output projection out_c = O_c @ W_o[rows of its heads]
Host side: shard/preprocess inputs (transpose x, slice + pre-scale weights),
then unshard by summing the 8 partial projections (the tensor-parallel
reduce) and adding b_o.

Schedule: the four batches are software-pipelined. While batch b's
attention runs (Act-engine exp is the per-tile critical path), batch
b+1's QKV projection matmuls and batch b's one-batch-lagged output
projection are drip-fed into the PE queue, so the tensor engine never
idles waiting for exp.  AV matmuls lag their exp by three k-tiles (slack
for the exp and for the av psum bank to rotate).  The k/v biases are
dropped (k's is softmax-invariant; v's folds into b_o host-side), exp
input is restricted to the live query range on diagonal tiles, and
batch 3 runs its chunks [1,2,3,0] so the two kept-back projection
chunks fill the PE during the final norm chain.

Problem constants (hardcoded per the harness contract):
  x [4, 2048, 1024] f32, W_qkv [1024, 3072], b_qkv [3072],
  W_o [1024, 1024], b_o [1024]; 16 heads, d_k = 64, causal.
"""

import numpy as np

N_CORES = 8
B, S, D = 4, 2048, 1024
H = 16
DK = 64
T = B * S            # 8192 tokens
HPC = H // N_CORES   # 2 heads per core
CPC = HPC * DK       # 128 channels per core per q/k/v
NB = S // 512        # 4 q-chunks of 512 per batch
NK = S // 128        # 16 k-tiles of 128 per batch

_PROFILE = False     # test.py may set kernel._PROFILE = True
_TRACE_DIR = None
_LAST_RESULT = None  # BassKernelResults of the last run (for test.py)

_PROGRAM = None      # cached program across calls

_QLO_RESTRICT = True  # restrict scores/exp to live query range on diag tiles
_DEBUG = False        # add den/rec/otn debug outputs
_PE_MASK = False    # causal mask via -1e9 PE accumulate (else DVE multiply)
_DMA_BCAST = False  # broadcast 1/den across partitions via DMA (unsupported)


def _build_program():
    import concourse.bacc as bacc
    import concourse.tile as tile
    from concourse import mybir

    F32 = mybir.dt.float32
    BF16 = mybir.dt.bfloat16
    AF = mybir.ActivationFunctionType

    nc = bacc.Bacc("TRN2", num_devices=N_CORES)

    # ---- DRAM parameters (per core) ----
    xT = nc.declare_dram_parameter("xT", [D, T], BF16, isOutput=False)
    wq = nc.declare_dram_parameter("wq", [128, 8 * CPC], BF16, isOutput=False)
    wk = nc.declare_dram_parameter("wk", [128, 8 * CPC], BF16, isOutput=False)
    wv = nc.declare_dram_parameter("wv", [128, 8 * CPC], BF16, isOutput=False)
    bq = nc.declare_dram_parameter("bq", [CPC, 1], F32, isOutput=False)
    wo = nc.declare_dram_parameter("wo", [CPC, D], BF16, isOutput=False)
    ident_d = nc.declare_dram_parameter("ident", [128, 128], BF16, isOutput=False)
    negI_d = nc.declare_dram_parameter("negI", [128, 128], BF16, isOutput=False)
    triu_d = nc.declare_dram_parameter("triu", [128, 2 * 128], BF16, isOutput=False)
    ones_d = nc.declare_dram_parameter("ones", [128, 1], BF16, isOutput=False)
    out = nc.declare_dram_parameter("out", [T, D], BF16, isOutput=True)
    if _DEBUG:
        dbg_den = nc.declare_dram_parameter("dbg_den", [32, 512], F32, isOutput=True)
        dbg_rec = nc.declare_dram_parameter("dbg_rec", [32, 512], F32, isOutput=True)
        dbg_otn = nc.declare_dram_parameter("dbg_otn", [CPC, T], BF16, isOutput=True)

    xT_t = xT.rearrange("(m p) t -> p m t", p=128)      # [128, 8, 8192]
    wq_t = wq.rearrange("p (m c) -> p m c", c=CPC)      # [128, 8, 128] contig
    wk_t = wk.rearrange("p (m c) -> p m c", c=CPC)
    wv_t = wv.rearrange("p (m c) -> p m c", c=CPC)

    with tile.TileContext(nc) as tc:
        with (
            tc.tile_pool(name="const", bufs=1) as const,
            tc.tile_pool(name="xt", bufs=4) as xt_pool,
            tc.tile_pool(name="qk", bufs=2) as qk_pool,
            tc.tile_pool(name="vt", bufs=3) as vt_pool,
            tc.tile_pool(name="pp", bufs=6) as pp_pool,
            tc.tile_pool(name="otn", bufs=3) as otn_pool,
            tc.tile_pool(name="osb", bufs=6) as osb_pool,
            tc.tile_pool(name="nrm", bufs=3) as nrm_pool,
            tc.tile_pool(name="ps", bufs=2, space="PSUM") as ps,
        ):
            # ---- PE p-state warmup: junk matmuls while the first DMAs are
            # in flight, so the clock is at full speed when real work lands.
            warm = const.tile([128, 640], BF16, tag="warm")
            nc.vector.memset(warm[:], 0.0)
            warm_ps = ps.tile([128, 2, 512], F32, tag="psw", name="warm_ps")
            for _ in range(8):
                nc.tensor.matmul(warm_ps[:, 0, :], warm[:, 0:128],
                                 warm[:, 128:640], start=True, stop=True)

            # ---- constants (x chunk 0 + wq are DMA'd first, below, so the
            # first q matmuls can start ASAP; the rest follow) ----
            wq_sb = const.tile([128, 8, CPC], BF16, tag="wq")
            wk_sb = const.tile([128, 8, CPC], BF16, tag="wk")
            wv_sb = const.tile([128, 8, CPC], BF16, tag="wv")
            bq_sb = const.tile([CPC, 1], F32, tag="bq")
            wo_sb = const.tile([CPC, D], BF16, tag="wo")
            ident = const.tile([128, 128], BF16, tag="ident")
            negI = const.tile([128, 128], BF16, tag="negI")
            triu = const.tile([128, 2, 128], BF16, tag="triu")
            ones_sb = const.tile([128, 1], BF16, tag="ones")

            def emit_const_dmas():
                nc.sync.dma_start(bq_sb[:], bq[:])
                nc.sync.dma_start(ident[:], ident_d[:])
                nc.sync.dma_start(negI[:], negI_d[:])
                nc.sync.dma_start(triu[:], triu_d.rearrange("p (g c) -> p g c", c=128))
                nc.sync.dma_start(ones_sb[:], ones_d[:])
                nc.sync.dma_start(wo_sb[:], wo[:])

            # persistent V^T tiles [V_A | 1 | 0pad | V_B | 1 | 0pad], two
            # sets (batch parity).  Padded to 128 weight columns per head so
            # the AV LDWEIGHTS qualifies for Fast Weight Load (NumWeights
            # must be 128); the junk av output rows 65:128 are never read.
            vaug = [
                [
                    const.tile([128, 2 * 128], BF16,
                               tag=f"vaug{st}_{j}", name=f"vaug{st}_{j}")
                    for j in range(NK)
                ]
                for st in range(2)
            ]
            def emit_persistent_init():
                # on GpSimd so the DVE is free for warmup QKV evictions
                for st in range(2):
                    for j in range(NK):
                        va_g = vaug[st][j].rearrange("p (g c) -> p g c", c=128)
                        nc.gpsimd.tensor_copy(
                            va_g[:, :, DK : DK + 1],
                            ones_sb[:, None, :].to_broadcast([128, 2, 1]))
                        nc.vector.memset(va_g[:, :, DK + 1 : 128], 0.0)

            qt_sb = {}
            kt_sb = {}
            otn = {}
            x_tiles = {}

            def prefetch_x(b2, ch, split=False):
                t0 = b2 * S
                c0 = ch * 512
                x_sb = xt_pool.tile([128, 8, 512], BF16, tag="x",
                                    name=f"x_{b2}_{ch}")
                sl = xT_t[:, :, t0 + c0 : t0 + c0 + 512]
                if split:
                    # two half-DMAs so the first m-groups can start as soon
                    # as the first half lands (startup latency)
                    nc.sync.dma_start(x_sb[:, 0:4, :], sl[:, 0:4, :])
                    nc.sync.dma_start(x_sb[:, 4:8, :], sl[:, 4:8, :])
                else:
                    nc.sync.dma_start(x_sb[:], sl)
                x_tiles[(b2, ch)] = x_sb

            def emit_qkv_units(b2, ch):
                """PE/DVE work units for QKV projection chunk (b2, ch)."""
                st = b2 % 2
                c0 = ch * 512
                units = []
                if ch == 0:
                    def alloc_qkt():
                        qt_sb[b2] = qk_pool.tile(
                            [CPC, S], BF16, tag="qt", name=f"qt_{b2}")
                        kt_sb[b2] = qk_pool.tile(
                            [CPC, S], BF16, tag="kt", name=f"kt_{b2}")
                    units.append(alloc_qkt)

                state = {}

                def take_x():
                    state["x"] = x_tiles.pop((b2, ch))
                units.append(take_x)

                def mm_group(w_sb, key, lo, hi):
                    def go():
                        if key not in state:
                            state[key] = ps.tile([CPC, 512], F32, tag="pk",
                                                 name=f"ps_{key}_{b2}_{ch}")
                        for m in range(lo, hi):
                            nc.tensor.matmul(state[key][:], w_sb[:, m, :],
                                             state["x"][:, m, :],
                                             start=(m == 0), stop=(m == 7))
                    return go

                def evict_q():
                    nc.vector.tensor_scalar_add(
                        qt_sb[b2][:, c0 : c0 + 512], state["q"][:], bq_sb[:])

                def evict_k():
                    # k bias dropped: it only adds a per-query constant to the
                    # scores, which softmax normalizes away.
                    nc.vector.tensor_copy(
                        kt_sb[b2][:, c0 : c0 + 512], state["k"][:])

                units.append(mm_group(wq_sb, "q", 0, 4))
                units.append(mm_group(wq_sb, "q", 4, 8))
                units.append(evict_q)
                units.append(mm_group(wk_sb, "k", 0, 4))
                units.append(mm_group(wk_sb, "k", 4, 8))
                units.append(evict_k)
                units.append(mm_group(wv_sb, "v", 0, 4))
                units.append(mm_group(wv_sb, "v", 4, 8))

                def evict_v():
                    # v bias dropped: softmax weights sum to 1, so b_v's
                    # contribution is a constant folded into b_o host-side.
                    vtmp = vt_pool.tile([CPC, 512], BF16, tag="vtmp",
                                        name=f"vtmp_{b2}_{ch}")
                    nc.vector.tensor_copy(vtmp[:], state["v"][:])
                    state["vtmp"] = vtmp
                units.append(evict_v)

                def transp(jj):
                    def go():
                        j = ch * 4 + jj
                        ps_t = ps.tile([128, 128], BF16, tag="pk",
                                       name=f"ps_t_{b2}_{ch}_{jj}")
                        nc.tensor.transpose(
                            ps_t[:], state["vtmp"][:, jj * 128 : jj * 128 + 128],
                            ident[:])
                        va_g = vaug[st][j].rearrange("p (g c) -> p g c", c=128)
                        pt_g = ps_t.rearrange("p (g c) -> p g c", c=DK)
                        if b2 == 0:
                            # warmup phase: DVE is busy with q/k/v evicts and
                            # Act is idle, so evict the transposes there
                            nc.scalar.copy(va_g[:, :, 0:DK], pt_g[:])
                        else:
                            nc.vector.tensor_copy(va_g[:, :, 0:DK], pt_g[:])
                    return go

                for jj in range(4):
                    units.append(transp(jj))
                return units

            def emit_proj_units(b, ch, act_evict=False):
                """Output projection for chunk (b, ch). bf16 eviction split
                across DVE and Act, one DMA per 128-token tile."""
                t0 = b * S
                c0 = ch * 512
                units = []

                def proj_mm(tt, state):
                    def go():
                        q0 = c0 + tt * 128
                        for half in range(2):
                            p_h = ps.tile([128, 512], F32, tag="pk",
                                          name=f"ps_o_{b}_{ch}_{tt}_{half}")
                            nc.tensor.matmul(p_h[:],
                                             otn[b][:, q0 : q0 + 128],
                                             wo_sb[:, half * 512 : half * 512 + 512],
                                             start=True, stop=True)
                            state[half] = p_h
                    return go

                def proj_out(tt, state):
                    def go():
                        q0 = c0 + tt * 128
                        o_sb = osb_pool.tile([128, D], BF16, tag="osb",
                                             name=f"o_sb_{b}_{ch}_{tt}")
                        if act_evict:
                            # tail chunks: both halves on Act, keeping the
                            # DVE free for the final norm chain
                            nc.scalar.copy(o_sb[:, 0:512], state[0][:])
                            nc.scalar.copy(o_sb[:, 512:1024], state[1][:])
                        else:
                            nc.vector.tensor_copy(o_sb[:, 0:512], state[0][:])
                            if b <= 1 or tt % 2 == 0:
                                nc.scalar.copy(o_sb[:, 512:1024], state[1][:])
                            else:
                                nc.vector.tensor_copy(o_sb[:, 512:1024], state[1][:])
                        nc.sync.dma_start(out[t0 + q0 : t0 + q0 + 128, :], o_sb[:])
                    return go

                for tt in range(4):
                    state = {}
                    units.append(proj_mm(tt, state))
                    units.append(proj_out(tt, state))
                return units

            def emit_attention(b, ch, units, tail_units=()):
                """Attention for chunk (b, ch), draining `units` between
                tiles; `tail_units` are emitted between the last AV and the
                norm so they fill the PE during the final norm chain."""
                st = b % 2
                c0 = ch * 512
                jmax = ch * 4 + 3
                ntiles = jmax + 1
                av_a = ps.tile([128, 512], F32, tag="av", name=f"av_a_{b}_{ch}")
                av_b = ps.tile([128, 512], F32, tag="av", name=f"av_b_{b}_{ch}")
                psel = [None] * ntiles
                plo = [0] * ntiles

                def emit_av(j):
                    first, last = j == 0, j == jmax
                    lo = plo[j]
                    nc.tensor.matmul(av_a[:, lo:512], vaug[st][j][:, 0:128],
                                     psel[j][:, 0, lo:512], start=first, stop=last)
                    nc.tensor.matmul(av_b[:, lo:512], vaug[st][j][:, 128:256],
                                     psel[j][:, 1, lo:512], start=first, stop=last)

                # distribute unit drains across tiles
                drains = [0] * (ntiles + 1)
                for u in range(len(units)):
                    drains[(u * ntiles) // len(units)] += 1
                ui = 0

                for j in range(ntiles):
                    r = j - ch * 4
                    k0 = j * 128
                    q_lo = (128 * r if r >= 1 else 0) if _QLO_RESTRICT else 0
                    diag = r >= 0
                    mmask = diag and _PE_MASK
                    s_ab = ps.tile([128, 2, 512], F32, tag="psw",
                                   name=f"s_ab_{b}_{ch}_{j}")
                    nc.tensor.matmul(
                        s_ab[:, 0, q_lo:512], kt_sb[b][0:DK, k0 : k0 + 128],
                        qt_sb[b][0:DK, c0 + q_lo : c0 + 512],
                        start=True, stop=not mmask, tile_position=(0, 0),
                    )
                    nc.tensor.matmul(
                        s_ab[:, 1, q_lo:512], kt_sb[b][DK:128, k0 : k0 + 128],
                        qt_sb[b][DK:128, c0 + q_lo : c0 + 512],
                        start=True, stop=not mmask, tile_position=(64, 0),
                    )
                    if mmask:
                        # causal mask: accumulate -1e9 onto the strictly-upper
                        # part of the 128-wide diagonal block so exp() zeroes
                        # it.  One full-K matmul per head plane: two row-tiled
                        # matmuls would accumulate into the same psum bank
                        # concurrently, which wedges the PE.
                        d0 = 128 * r
                        for h in range(2):
                            nc.tensor.matmul(
                                s_ab[:, h, d0 : d0 + 128], negI[:],
                                triu[:, h, :],
                                start=False, stop=True,
                            )
                    psel[j] = pp_pool.tile([128, 2, 512], BF16, tag="pt",
                                           name=f"p_{b}_{ch}_{j}")
                    plo[j] = q_lo
                    nc.scalar.activation(psel[j][:, :, q_lo:512],
                                         s_ab[:, :, q_lo:512], AF.Exp)
                    if diag and not _PE_MASK:
                        d0 = 128 * r
                        pm = psel[j][:, :, d0 : d0 + 128]
                        nc.vector.tensor_mul(pm, pm, triu[:])
                    if j >= 3:
                        # three-tile lag: ample slack for exp, and the first
                        # AV lands after the previous chunk's norm has freed
                        # the av psum bank
                        emit_av(j - 3)
                    for _ in range(drains[j]):
                        units[ui]()
                        ui += 1
                for jj in range(max(0, ntiles - 3), ntiles):
                    emit_av(jj)
                for u in tail_units:
                    u()

                # normalize: otn[h] = av[0:64] * (1 / av[64]).
                # reciprocal_approx_fast must NOT read PSUM directly, and
                # needs a partition-0 input (custom-DVE op): stage den
                # through a [1,512] SBUF tile.
                def norm(hh, av, cl, w):
                    den = nrm_pool.tile([1, 512], F32, tag="den",
                                        name=f"den_{b}_{ch}_{hh}_{cl}")
                    nc.vector.tensor_copy(den[:, 0:w], av[DK : DK + 1, cl : cl + w])
                    rec = nrm_pool.tile([1, 512], F32, tag="rec",
                                        name=f"rec_{b}_{ch}_{hh}_{cl}")
                    nc.vector.reciprocal_approx_fast(rec[:, 0:w], den[:, 0:w])
                    bc = nrm_pool.tile([DK, 512], F32, tag="bc",
                                       name=f"bc_{b}_{ch}_{hh}_{cl}")
                    nc.gpsimd.partition_broadcast(bc[:, 0:w], rec[:, 0:w])
                    nc.vector.tensor_mul(
                        otn[b][hh * DK : hh * DK + DK, c0 + cl : c0 + cl + w],
                        av[0:DK, cl : cl + w], bc[:, 0:w],
                    )

                for hh, av in ((0, av_a), (1, av_b)):
                    norm(hh, av, 0, 512)
                if _DEBUG and ch == NB - 1:
                    nc.sync.dma_start(dbg_otn[:, b * S : (b + 1) * S], otn[b][:])
                while ui < len(units):
                    units[ui]()
                    ui += 1

            # x-chunk prefetch: one slot of lead over the QKV consumption.
            # QKV consumption order: (0,0) in the (short) warmup, then batch
            # 0's remaining chunks ride inside the batch-0 attention slots
            # alongside batch 1's.
            seq = [(0, c) for c in range(NB)] + [
                (b + 1, c) for b in range(B - 1) for c in range(NB)]
            pf = {"i": 1}

            def pump(n=1):
                for _ in range(n):
                    if pf["i"] < len(seq):
                        prefetch_x(*seq[pf["i"]])
                        pf["i"] += 1

            # ---- warmup: QKV for batch 0 ----
            # DMA order: x(0,0) first half, wq, x(0,0) second half, wk, wv,
            # then the next x chunks — so the first q matmuls are unblocked
            # as early as possible.  The V transposes of each chunk are
            # deferred past the next chunk's q matmuls so the PE never waits
            # on the DVE eviction chain.
            # x halves on the SP queue, weights on the Act queue: dispatches
            # and transfers run in parallel on separate DMA queues
            x00 = xt_pool.tile([128, 8, 512], BF16, tag="x", name="x_0_0")
            nc.sync.dma_start(x00[:, 0:4, :], xT_t[:, 0:4, 0:512])
            nc.scalar.dma_start(wq_sb[:], wq_t)
            nc.sync.dma_start(x00[:, 4:8, :], xT_t[:, 4:8, 0:512])
            nc.scalar.dma_start(wk_sb[:], wk_t)
            nc.scalar.dma_start(wv_sb[:], wv_t)
            x_tiles[(0, 0)] = x00
            pump(2)
            prev_tr = []
            for ch in range(NB):
                us = emit_qkv_units(0, ch)
                main, tr = us[:-4], us[-4:]
                for i, u in enumerate(main):
                    u()
                    if ch == 0 and i == 0:
                        emit_const_dmas()
                    if i == 3 and prev_tr:
                        # previous chunk's V transposes after this chunk's
                        # first q matmuls, so the PE isn't stuck waiting on
                        # the DVE eviction chain
                        for t in prev_tr:
                            t()
                        prev_tr = []
                if ch == 0:
                    emit_persistent_init()
                else:
                    pump()
                prev_tr = tr
            for t in prev_tr:
                t()

            # ---- pipelined main loop ----
            # proj chunks go through a FIFO lagged one full batch: batch b's
            # projections drain inside batch b+1's slots.  b=3's slots carry 2
            # chunks each (they have no QKV work left to keep the PE busy),
            # leaving a single chunk for the tail drain.
            pending = []
            for b in range(B):
                otn[b] = otn_pool.tile([CPC, S], BF16, tag="otn", name=f"otn_{b}")
                # b3 runs its chunks [1,2,3,0] so the final chunk is the
                # small one and the (3,3) projection can fill the PE during
                # the last norm chain
                chs = [1, 2, 3, 0] if b == B - 1 else range(NB)
                for k, ch in enumerate(chs):
                    last = b == B - 1 and k == NB - 1
                    units = []
                    if b + 1 < B:
                        units += emit_qkv_units(b + 1, ch)
                        pump()
                    ndrain = {0: 0, 1: 1, 2: 1, 3: 2}[b]
                    if b == B - 1:
                        # keep two chunks back as PE filler for the final
                        # norm chain
                        ndrain = [2, 2, 1, 0][k]
                    for _ in range(ndrain):
                        if pending:
                            units += emit_proj_units(*pending.pop(0))
                    tail_units = []
                    if last:
                        while pending:
                            tail_units += emit_proj_units(*pending.pop(0),
                                                          act_evict=True)
                    emit_attention(b, ch, units, tail_units)
                    pending.append((b, ch))
            while pending:
                for u in emit_proj_units(*pending.pop(0)):
                    u()

    nc.compile()
    return nc


def _get_program():
    global _PROGRAM
    if _PROGRAM is None:
        _PROGRAM = _build_program()
    return _PROGRAM


def kernel(x, W_qkv, b_qkv, W_o, b_o):
    global _LAST_RESULT
    from concourse.bass_utils import run_bass_kernel_spmd

    x = np.asarray(x, np.float32)
    W_qkv = np.asarray(W_qkv, np.float32)
    b_qkv = np.asarray(b_qkv, np.float32)
    W_o = np.asarray(W_o, np.float32)
    b_o = np.asarray(b_o, np.float32)

    # host-side shard/preprocess
    import ml_dtypes
    bf16 = ml_dtypes.bfloat16
    xT = np.ascontiguousarray(x.reshape(T, D).T).astype(bf16)   # [1024, 8192]
    scale = np.float32(1.0 / np.sqrt(DK))
    ones = np.ones((128, 1), bf16)
    ident = np.eye(128, dtype=bf16)
    # PE-side causal mask: negI @ triu accumulates -1e9 where key > query
    # on the 128-wide diagonal block (pattern duplicated for the two heads).
    # With _PE_MASK off, "triu" instead carries the keep-mask for the DVE
    # multiply fallback.
    negI = (np.float32(-1e9) * np.eye(128, dtype=np.float32)).astype(bf16)
    cmp = np.arange(128)[:, None] > np.arange(128)[None, :]
    triu1 = (cmp if _PE_MASK else ~cmp).astype(bf16)
    triu = np.ascontiguousarray(np.concatenate([triu1, triu1], axis=1))

    def _wprep(w):
        # [1024, 128] -> [128, 8*128]: per-partition contiguous DMA layout
        return np.ascontiguousarray(
            w.reshape(8, 128, CPC).transpose(1, 0, 2).reshape(128, 8 * CPC)
        ).astype(bf16)

    in_maps = []
    for c in range(N_CORES):
        cs = c * CPC
        in_maps.append({
            "xT": xT,
            "wq": _wprep(W_qkv[:, cs : cs + CPC] * scale),
            "wk": _wprep(W_qkv[:, D + cs : D + cs + CPC]),
            "wv": _wprep(W_qkv[:, 2 * D + cs : 2 * D + cs + CPC]),
            "bq": np.ascontiguousarray(b_qkv[cs : cs + CPC, None] * scale),
            "wo": np.ascontiguousarray(W_o[cs : cs + CPC, :]).astype(bf16),
            "ident": ident,
            "negI": negI,
            "triu": triu,
            "ones": ones,
        })

    nc = _get_program()
    res = run_bass_kernel_spmd(
        nc, in_maps, list(range(N_CORES)),
        trace=_PROFILE, tmpdir=_TRACE_DIR,
    )
    _LAST_RESULT = res

    # unshard: tensor-parallel reduce of the 8 partial projections, plus
    # b_o and the folded v-bias contribution (softmax weights sum to 1, so
    # the attention output of v + b_v is the output of v plus b_v exactly).
    acc = res.results[0]["out"].astype(np.float32)
    for c in range(1, N_CORES):
        acc += res.results[c]["out"]
    acc += b_o[None, :] + b_qkv[2 * D : 3 * D] @ W_o
    return acc.reshape(B, S, D)



# revision 51
# speedup vs baseline: 1.1010x; 1.0043x over previous
"""Causal multi-head attention on 8 Trainium2 NeuronCores.

Sharding: tensor parallel over heads. Core c owns heads {2c, 2c+1}:
  - QKV projection for its 128 q / 128 k / 128 v channels, all B*S tokens
  - causal attention for its 2 heads (row-packed K=64 matmuls)
  - partial output projection out_c = O_c @ W_o[rows of its heads]
Host side: shard/preprocess inputs (transpose x, slice + pre-scale weights),
then unshard by summing the 8 partial projections (the tensor-parallel
reduce) and adding b_o.

Schedule: the four batches are software-pipelined. While batch b's
attention runs (Act-engine exp is the per-tile critical path), batch
b+1's QKV projection matmuls and batch b's one-batch-lagged output
projection are drip-fed into the PE queue, so the tensor engine never
idles waiting for exp.  AV matmuls lag their exp by three k-tiles (slack
for the exp and for the av psum bank to rotate).  The k/v biases are
dropped (k's is softmax-invariant; v's folds into b_o host-side), exp
input is restricted to the live query range on diagonal tiles, and
batch 3 runs its chunks [1,2,3,0] so the two kept-back projection
chunks fill the PE during the final norm chain.

Problem constants (hardcoded per the harness contract):
  x [4, 2048, 1024] f32, W_qkv [1024, 3072], b_qkv [3072],
  W_o [1024, 1024], b_o [1024]; 16 heads, d_k = 64, causal.
"""

import numpy as np

N_CORES = 8
B, S, D = 4, 2048, 1024
H = 16
DK = 64
T = B * S            # 8192 tokens
HPC = H // N_CORES   # 2 heads per core
CPC = HPC * DK       # 128 channels per core per q/k/v
NB = S // 512        # 4 q-chunks of 512 per batch
NK = S // 128        # 16 k-tiles of 128 per batch

_PROFILE = False     # test.py may set kernel._PROFILE = True
_TRACE_DIR = None
_LAST_RESULT = None  # BassKernelResults of the last run (for test.py)

_PROGRAM = None      # cached program across calls

_QLO_RESTRICT = True  # restrict scores/exp to live query range on diag tiles
_DEBUG = False        # add den/rec/otn debug outputs
_PE_MASK = False    # causal mask via -1e9 PE accumulate (else DVE multiply)
_DMA_BCAST = False  # broadcast 1/den across partitions via DMA (unsupported)


def _build_program():
    import concourse.bacc as bacc
    import concourse.tile as tile
    from concourse import mybir

    F32 = mybir.dt.float32
    BF16 = mybir.dt.bfloat16
    AF = mybir.ActivationFunctionType

    nc = bacc.Bacc("TRN2", num_devices=N_CORES)

    # ---- DRAM parameters (per core) ----
    xT = nc.declare_dram_parameter("xT", [D, T], BF16, isOutput=False)
    wq = nc.declare_dram_parameter("wq", [128, 8 * CPC], BF16, isOutput=False)
    wk = nc.declare_dram_parameter("wk", [128, 8 * CPC], BF16, isOutput=False)
    wv = nc.declare_dram_parameter("wv", [128, 8 * CPC], BF16, isOutput=False)
    bq = nc.declare_dram_parameter("bq", [CPC, 1], F32, isOutput=False)
    wo = nc.declare_dram_parameter("wo", [CPC, D], BF16, isOutput=False)
    ident_d = nc.declare_dram_parameter("ident", [128, 128], BF16, isOutput=False)
    negI_d = nc.declare_dram_parameter("negI", [128, 128], BF16, isOutput=False)
    triu_d = nc.declare_dram_parameter("triu", [128, 2 * 128], BF16, isOutput=False)
    ones_d = nc.declare_dram_parameter("ones", [128, 1], BF16, isOutput=False)
    out = nc.declare_dram_parameter("out", [T, D], BF16, isOutput=True)
    if _DEBUG:
        dbg_den = nc.declare_dram_parameter("dbg_den", [32, 512], F32, isOutput=True)
        dbg_rec = nc.declare_dram_parameter("dbg_rec", [32, 512], F32, isOutput=True)
        dbg_otn = nc.declare_dram_parameter("dbg_otn", [CPC, T], BF16, isOutput=True)

    xT_t = xT.rearrange("(m p) t -> p m t", p=128)      # [128, 8, 8192]
    wq_t = wq.rearrange("p (m c) -> p m c", c=CPC)      # [128, 8, 128] contig
    wk_t = wk.rearrange("p (m c) -> p m c", c=CPC)
    wv_t = wv.rearrange("p (m c) -> p m c", c=CPC)

    with tile.TileContext(nc) as tc:
        with (
            tc.tile_pool(name="const", bufs=1) as const,
            tc.tile_pool(name="xt", bufs=4) as xt_pool,
            tc.tile_pool(name="qk", bufs=2) as qk_pool,
            tc.tile_pool(name="vt", bufs=3) as vt_pool,
            tc.tile_pool(name="pp", bufs=6) as pp_pool,
            tc.tile_pool(name="otn", bufs=3) as otn_pool,
            tc.tile_pool(name="osb", bufs=6) as osb_pool,
            tc.tile_pool(name="nrm", bufs=3) as nrm_pool,
            tc.tile_pool(name="ps", bufs=2, space="PSUM") as ps,
        ):
            # ---- PE p-state warmup: junk matmuls while the first DMAs are
            # in flight, so the clock is at full speed when real work lands.
            warm = const.tile([128, 640], BF16, tag="warm")
            nc.vector.memset(warm[:], 0.0)
            warm_ps = ps.tile([128, 2, 512], F32, tag="psw", name="warm_ps")
            for _ in range(8):
                nc.tensor.matmul(warm_ps[:, 0, :], warm[:, 0:128],
                                 warm[:, 128:640], start=True, stop=True)

            # ---- constants (x chunk 0 + wq are DMA'd first, below, so the
            # first q matmuls can start ASAP; the rest follow) ----
            wq_sb = const.tile([128, 8, CPC], BF16, tag="wq")
            wk_sb = const.tile([128, 8, CPC], BF16, tag="wk")
            wv_sb = const.tile([128, 8, CPC], BF16, tag="wv")
            bq_sb = const.tile([CPC, 1], F32, tag="bq")
            wo_sb = const.tile([CPC, D], BF16, tag="wo")
            ident = const.tile([128, 128], BF16, tag="ident")
            negI = const.tile([128, 128], BF16, tag="negI")
            triu = const.tile([128, 2, 128], BF16, tag="triu")
            ones_sb = const.tile([128, 1], BF16, tag="ones")

            def emit_const_dmas():
                nc.sync.dma_start(bq_sb[:], bq[:])
                nc.sync.dma_start(ident[:], ident_d[:])
                nc.sync.dma_start(negI[:], negI_d[:])
                nc.sync.dma_start(triu[:], triu_d.rearrange("p (g c) -> p g c", c=128))
                nc.sync.dma_start(ones_sb[:], ones_d[:])
                nc.sync.dma_start(wo_sb[:], wo[:])

            # persistent V^T tiles [V_A | 1 | 0pad | V_B | 1 | 0pad], two
            # sets (batch parity).  Padded to 128 weight columns per head so
            # the AV LDWEIGHTS qualifies for Fast Weight Load (NumWeights
            # must be 128); the junk av output rows 65:128 are never read.
            vaug = [
                [
                    const.tile([128, 2 * 128], BF16,
                               tag=f"vaug{st}_{j}", name=f"vaug{st}_{j}")
                    for j in range(NK)
                ]
                for st in range(2)
            ]
            def emit_persistent_init():
                # on GpSimd so the DVE is free for warmup QKV evictions
                for st in range(2):
                    for j in range(NK):
                        va_g = vaug[st][j].rearrange("p (g c) -> p g c", c=128)
                        nc.gpsimd.tensor_copy(
                            va_g[:, :, DK : DK + 1],
                            ones_sb[:, None, :].to_broadcast([128, 2, 1]))
                        nc.vector.memset(va_g[:, :, DK + 1 : 128], 0.0)

            qt_sb = {}
            kt_sb = {}
            otn = {}
            x_tiles = {}

            def prefetch_x(b2, ch, split=False):
                t0 = b2 * S
                c0 = ch * 512
                x_sb = xt_pool.tile([128, 8, 512], BF16, tag="x",
                                    name=f"x_{b2}_{ch}")
                sl = xT_t[:, :, t0 + c0 : t0 + c0 + 512]
                if split:
                    # two half-DMAs so the first m-groups can start as soon
                    # as the first half lands (startup latency)
                    nc.sync.dma_start(x_sb[:, 0:4, :], sl[:, 0:4, :])
                    nc.sync.dma_start(x_sb[:, 4:8, :], sl[:, 4:8, :])
                else:
                    nc.sync.dma_start(x_sb[:], sl)
                x_tiles[(b2, ch)] = x_sb

            def emit_qkv_units(b2, ch):
                """PE/DVE work units for QKV projection chunk (b2, ch)."""
                st = b2 % 2
                c0 = ch * 512
                units = []
                if ch == 0:
                    def alloc_qkt():
                        qt_sb[b2] = qk_pool.tile(
                            [CPC, S], BF16, tag="qt", name=f"qt_{b2}")
                        kt_sb[b2] = qk_pool.tile(
                            [CPC, S], BF16, tag="kt", name=f"kt_{b2}")
                    units.append(alloc_qkt)

                state = {}

                def take_x():
                    state["x"] = x_tiles.pop((b2, ch))
                units.append(take_x)

                def mm_group(w_sb, key, lo, hi):
                    def go():
                        if key not in state:
                            state[key] = ps.tile([CPC, 512], F32, tag="pk",
                                                 name=f"ps_{key}_{b2}_{ch}")
                        for m in range(lo, hi):
                            nc.tensor.matmul(state[key][:], w_sb[:, m, :],
                                             state["x"][:, m, :],
                                             start=(m == 0), stop=(m == 7))
                    return go

                def evict_q():
                    nc.vector.tensor_scalar_add(
                        qt_sb[b2][:, c0 : c0 + 512], state["q"][:], bq_sb[:])

                def evict_k():
                    # k bias dropped: it only adds a per-query constant to the
                    # scores, which softmax normalizes away.
                    nc.vector.tensor_copy(
                        kt_sb[b2][:, c0 : c0 + 512], state["k"][:])

                units.append(mm_group(wq_sb, "q", 0, 4))
                units.append(mm_group(wq_sb, "q", 4, 8))
                units.append(evict_q)
                units.append(mm_group(wk_sb, "k", 0, 4))
                units.append(mm_group(wk_sb, "k", 4, 8))
                units.append(evict_k)
                units.append(mm_group(wv_sb, "v", 0, 4))
                units.append(mm_group(wv_sb, "v", 4, 8))

                def evict_v():
                    # v bias dropped: softmax weights sum to 1, so b_v's
                    # contribution is a constant folded into b_o host-side.
                    vtmp = vt_pool.tile([CPC, 512], BF16, tag="vtmp",
                                        name=f"vtmp_{b2}_{ch}")
                    nc.vector.tensor_copy(vtmp[:], state["v"][:])
                    state["vtmp"] = vtmp
                units.append(evict_v)

                def transp(jj):
                    def go():
                        j = ch * 4 + jj
                        ps_t = ps.tile([128, 128], BF16, tag="pk",
                                       name=f"ps_t_{b2}_{ch}_{jj}")
                        nc.tensor.transpose(
                            ps_t[:], state["vtmp"][:, jj * 128 : jj * 128 + 128],
                            ident[:])
                        va_g = vaug[st][j].rearrange("p (g c) -> p g c", c=128)
                        pt_g = ps_t.rearrange("p (g c) -> p g c", c=DK)
                        if b2 == 0:
                            # warmup phase: DVE is busy with q/k/v evicts and
                            # Act is idle, so evict the transposes there
                            nc.scalar.copy(va_g[:, :, 0:DK], pt_g[:])
                        else:
                            nc.vector.tensor_copy(va_g[:, :, 0:DK], pt_g[:])
                    return go

                for jj in range(4):
                    units.append(transp(jj))
                return units

            def emit_proj_units(b, ch, act_evict=False):
                """Output projection for chunk (b, ch). bf16 eviction split
                across DVE and Act, one DMA per 128-token tile."""
                t0 = b * S
                c0 = ch * 512
                units = []

                def proj_mm(tt, state):
                    def go():
                        q0 = c0 + tt * 128
                        for half in range(2):
                            p_h = ps.tile([128, 512], F32, tag="pk",
                                          name=f"ps_o_{b}_{ch}_{tt}_{half}")
                            nc.tensor.matmul(p_h[:],
                                             otn[b][:, q0 : q0 + 128],
                                             wo_sb[:, half * 512 : half * 512 + 512],
                                             start=True, stop=True)
                            state[half] = p_h
                    return go

                def proj_out(tt, state):
                    def go():
                        q0 = c0 + tt * 128
                        o_sb = osb_pool.tile([128, D], BF16, tag="osb",
                                             name=f"o_sb_{b}_{ch}_{tt}")
                        if act_evict:
                            # tail chunks: both halves on Act, keeping the
                            # DVE free for the final norm chain
                            nc.scalar.copy(o_sb[:, 0:512], state[0][:])
                            nc.scalar.copy(o_sb[:, 512:1024], state[1][:])
                        else:
                            nc.vector.tensor_copy(o_sb[:, 0:512], state[0][:])
                            if b <= 1 or tt % 2 == 0:
                                nc.scalar.copy(o_sb[:, 512:1024], state[1][:])
                            else:
                                nc.vector.tensor_copy(o_sb[:, 512:1024], state[1][:])
                        nc.sync.dma_start(out[t0 + q0 : t0 + q0 + 128, :], o_sb[:])
                    return go

                for tt in range(4):
                    state = {}
                    units.append(proj_mm(tt, state))
                    units.append(proj_out(tt, state))
                return units

            def emit_attention(b, ch, units, tail_units=()):
                """Attention for chunk (b, ch), draining `units` between
                tiles; `tail_units` are emitted between the last AV and the
                norm so they fill the PE during the final norm chain."""
                st = b % 2
                c0 = ch * 512
                jmax = ch * 4 + 3
                ntiles = jmax + 1
                av_a = ps.tile([128, 512], F32, tag="av", name=f"av_a_{b}_{ch}")
                av_b = ps.tile([128, 512], F32, tag="av", name=f"av_b_{b}_{ch}")
                psel = [None] * ntiles
                plo = [0] * ntiles

                def emit_av(j):
                    first, last = j == 0, j == jmax
                    lo = plo[j]
                    nc.tensor.matmul(av_a[:, lo:512], vaug[st][j][:, 0:128],
                                     psel[j][:, 0, lo:512], start=first, stop=last)
                    nc.tensor.matmul(av_b[:, lo:512], vaug[st][j][:, 128:256],
                                     psel[j][:, 1, lo:512], start=first, stop=last)

                # distribute unit drains across tiles
                drains = [0] * (ntiles + 1)
                for u in range(len(units)):
                    drains[(u * ntiles) // len(units)] += 1
                ui = 0

                for j in range(ntiles):
                    r = j - ch * 4
                    k0 = j * 128
                    q_lo = (128 * r if r >= 1 else 0) if _QLO_RESTRICT else 0
                    diag = r >= 0
                    mmask = diag and _PE_MASK
                    s_ab = ps.tile([128, 2, 512], F32, tag="psw",
                                   name=f"s_ab_{b}_{ch}_{j}")
                    nc.tensor.matmul(
                        s_ab[:, 0, q_lo:512], kt_sb[b][0:DK, k0 : k0 + 128],
                        qt_sb[b][0:DK, c0 + q_lo : c0 + 512],
                        start=True, stop=not mmask, tile_position=(0, 0),
                    )
                    nc.tensor.matmul(
                        s_ab[:, 1, q_lo:512], kt_sb[b][DK:128, k0 : k0 + 128],
                        qt_sb[b][DK:128, c0 + q_lo : c0 + 512],
                        start=True, stop=not mmask, tile_position=(64, 0),
                    )
                    if mmask:
                        # causal mask: accumulate -1e9 onto the strictly-upper
                        # part of the 128-wide diagonal block so exp() zeroes
                        # it.  One full-K matmul per head plane: two row-tiled
                        # matmuls would accumulate into the same psum bank
                        # concurrently, which wedges the PE.
                        d0 = 128 * r
                        for h in range(2):
                            nc.tensor.matmul(
                                s_ab[:, h, d0 : d0 + 128], negI[:],
                                triu[:, h, :],
                                start=False, stop=True,
                            )
                    psel[j] = pp_pool.tile([128, 2, 512], BF16, tag="pt",
                                           name=f"p_{b}_{ch}_{j}")
                    plo[j] = q_lo
                    nc.scalar.activation(psel[j][:, :, q_lo:512],
                                         s_ab[:, :, q_lo:512], AF.Exp)
                    if diag and not _PE_MASK:
                        d0 = 128 * r
                        pm = psel[j][:, :, d0 : d0 + 128]
                        nc.vector.tensor_mul(pm, pm, triu[:])
                    # lagged AV: ample slack for exp, and the first AV
                    # lands after the previous chunk's norm has freed the av
                    # psum bank.  b3 slots have no QKV drip work, so their
                    # first AV arrives sooner and needs one more tile of lag.
                    lag = 4 if b == B - 1 else 3
                    if j >= lag:
                        emit_av(j - lag)
                    for _ in range(drains[j]):
                        units[ui]()
                        ui += 1
                for jj in range(max(0, ntiles - lag), ntiles):
                    emit_av(jj)
                for u in tail_units:
                    u()

                # normalize: otn[h] = av[0:64] * (1 / av[64]).
                # reciprocal_approx_fast must NOT read PSUM directly, and
                # needs a partition-0 input (custom-DVE op): stage den
                # through a [1,512] SBUF tile.
                def norm(hh, av, cl, w):
                    den = nrm_pool.tile([1, 512], F32, tag="den",
                                        name=f"den_{b}_{ch}_{hh}_{cl}")
                    nc.vector.tensor_copy(den[:, 0:w], av[DK : DK + 1, cl : cl + w])
                    rec = nrm_pool.tile([1, 512], F32, tag="rec",
                                        name=f"rec_{b}_{ch}_{hh}_{cl}")
                    nc.vector.reciprocal_approx_fast(rec[:, 0:w], den[:, 0:w])
                    bc = nrm_pool.tile([DK, 512], F32, tag="bc",
                                       name=f"bc_{b}_{ch}_{hh}_{cl}")
                    nc.gpsimd.partition_broadcast(bc[:, 0:w], rec[:, 0:w])
                    nc.vector.tensor_mul(
                        otn[b][hh * DK : hh * DK + DK, c0 + cl : c0 + cl + w],
                        av[0:DK, cl : cl + w], bc[:, 0:w],
                    )

                for hh, av in ((0, av_a), (1, av_b)):
                    norm(hh, av, 0, 512)
                if _DEBUG and ch == NB - 1:
                    nc.sync.dma_start(dbg_otn[:, b * S : (b + 1) * S], otn[b][:])
                while ui < len(units):
                    units[ui]()
                    ui += 1

            # x-chunk prefetch: one slot of lead over the QKV consumption.
            # QKV consumption order: (0,0) in the (short) warmup, then batch
            # 0's remaining chunks ride inside the batch-0 attention slots
            # alongside batch 1's.
            seq = [(0, c) for c in range(NB)] + [
                (b + 1, c) for b in range(B - 1) for c in range(NB)]
            pf = {"i": 1}

            def pump(n=1):
                for _ in range(n):
                    if pf["i"] < len(seq):
                        prefetch_x(*seq[pf["i"]])
                        pf["i"] += 1

            # ---- warmup: QKV for batch 0 ----
            # DMA order: x(0,0) first half, wq, x(0,0) second half, wk, wv,
            # then the next x chunks — so the first q matmuls are unblocked
            # as early as possible.  The V transposes of each chunk are
            # deferred past the next chunk's q matmuls so the PE never waits
            # on the DVE eviction chain.
            # x halves on the SP queue, weights on the Act queue: dispatches
            # and transfers run in parallel on separate DMA queues
            x00 = xt_pool.tile([128, 8, 512], BF16, tag="x", name="x_0_0")
            nc.sync.dma_start(x00[:, 0:4, :], xT_t[:, 0:4, 0:512])
            nc.scalar.dma_start(wq_sb[:], wq_t)
            nc.sync.dma_start(x00[:, 4:8, :], xT_t[:, 4:8, 0:512])
            nc.scalar.dma_start(wk_sb[:], wk_t)
            nc.scalar.dma_start(wv_sb[:], wv_t)
            x_tiles[(0, 0)] = x00
            pump(2)
            prev_tr = []
            for ch in range(NB):
                us = emit_qkv_units(0, ch)
                main, tr = us[:-4], us[-4:]
                for i, u in enumerate(main):
                    u()
                    if ch == 0 and i == 0:
                        emit_const_dmas()
                    if i == 3 and prev_tr:
                        # previous chunk's V transposes after this chunk's
                        # first q matmuls, so the PE isn't stuck waiting on
                        # the DVE eviction chain
                        for t in prev_tr:
                            t()
                        prev_tr = []
                if ch == 0:
                    emit_persistent_init()
                else:
                    pump()
                prev_tr = tr
            for t in prev_tr:
                t()

            # ---- pipelined main loop ----
            # proj chunks go through a FIFO lagged one full batch: batch b's
            # projections drain inside batch b+1's slots.  b=3's slots carry 2
            # chunks each (they have no QKV work left to keep the PE busy),
            # leaving a single chunk for the tail drain.
            pending = []
            for b in range(B):
                otn[b] = otn_pool.tile([CPC, S], BF16, tag="otn", name=f"otn_{b}")
                # b3 runs its chunks [1,2,3,0] so the final chunk is the
                # small one and the (3,3) projection can fill the PE during
                # the last norm chain
                chs = [1, 2, 3, 0] if b == B - 1 else range(NB)
                for k, ch in enumerate(chs):
                    last = b == B - 1 and k == NB - 1
                    units = []
                    if b + 1 < B:
                        units += emit_qkv_units(b + 1, ch)
                        pump()
                    ndrain = {0: 0, 1: 1, 2: 1, 3: 2}[b]
                    if b == B - 1:
                        # keep two chunks back as PE filler for the final
                        # norm chain
                        ndrain = [2, 2, 1, 0][k]
                    for _ in range(ndrain):
                        if pending:
                            units += emit_proj_units(*pending.pop(0))
                    tail_units = []
                    if last:
                        while pending:
                            tail_units += emit_proj_units(*pending.pop(0),
                                                          act_evict=True)
                    emit_attention(b, ch, units, tail_units)
                    pending.append((b, ch))
            while pending:
                for u in emit_proj_units(*pending.pop(0)):
                    u()

    nc.compile()
    return nc


def _get_program():
    global _PROGRAM
    if _PROGRAM is None:
        _PROGRAM = _build_program()
    return _PROGRAM


def kernel(x, W_qkv, b_qkv, W_o, b_o):
    global _LAST_RESULT
    from concourse.bass_utils import run_bass_kernel_spmd

    x = np.asarray(x, np.float32)
    W_qkv = np.asarray(W_qkv, np.float32)
    b_qkv = np.asarray(b_qkv, np.float32)
    W_o = np.asarray(W_o, np.float32)
    b_o = np.asarray(b_o, np.float32)

    # host-side shard/preprocess
    import ml_dtypes
    bf16 = ml_dtypes.bfloat16
    xT = np.ascontiguousarray(x.reshape(T, D).T).astype(bf16)   # [1024, 8192]
    scale = np.float32(1.0 / np.sqrt(DK))
    ones = np.ones((128, 1), bf16)
    ident = np.eye(128, dtype=bf16)
    # PE-side causal mask: negI @ triu accumulates -1e9 where key > query
    # on the 128-wide diagonal block (pattern duplicated for the two heads).
    # With _PE_MASK off, "triu" instead carries the keep-mask for the DVE
    # multiply fallback.
    negI = (np.float32(-1e9) * np.eye(128, dtype=np.float32)).astype(bf16)
    cmp = np.arange(128)[:, None] > np.arange(128)[None, :]
    triu1 = (cmp if _PE_MASK else ~cmp).astype(bf16)
    triu = np.ascontiguousarray(np.concatenate([triu1, triu1], axis=1))

    def _wprep(w):
        # [1024, 128] -> [128, 8*128]: per-partition contiguous DMA layout
        return np.ascontiguousarray(
            w.reshape(8, 128, CPC).transpose(1, 0, 2).reshape(128, 8 * CPC)
        ).astype(bf16)

    in_maps = []
    for c in range(N_CORES):
        cs = c * CPC
        in_maps.append({
            "xT": xT,
            "wq": _wprep(W_qkv[:, cs : cs + CPC] * scale),
            "wk": _wprep(W_qkv[:, D + cs : D + cs + CPC]),
            "wv": _wprep(W_qkv[:, 2 * D + cs : 2 * D + cs + CPC]),
            "bq": np.ascontiguousarray(b_qkv[cs : cs + CPC, None] * scale),
            "wo": np.ascontiguousarray(W_o[cs : cs + CPC, :]).astype(bf16),
            "ident": ident,
            "negI": negI,
            "triu": triu,
            "ones": ones,
        })

    nc = _get_program()
    res = run_bass_kernel_spmd(
        nc, in_maps, list(range(N_CORES)),
        trace=_PROFILE, tmpdir=_TRACE_DIR,
    )
    _LAST_RESULT = res

    # unshard: tensor-parallel reduce of the 8 partial projections, plus
    # b_o and the folded v-bias contribution (softmax weights sum to 1, so
    # the attention output of v + b_v is the output of v plus b_v exactly).
    acc = res.results[0]["out"].astype(np.float32)
    for c in range(1, N_CORES):
        acc += res.results[c]["out"]
    acc += b_o[None, :] + b_qkv[2 * D : 3 * D] @ W_o
    return acc.reshape(B, S, D)

